# revision 45
# baseline (speedup 1.0000x reference)
"""Trainium2 Bass kernel for nn_Attention_37847251812733.

Full transformer block: QKV proj -> 16-head attention (N=4096, DH=64)
-> permuted reshape (the reference's transpose(1,2).reshape) -> LN ->
MLP -> LN.  Tensor-parallel over heads; core c owns heads {2c, 2c+1}
and produces rows [512c, 512c+512) of the permuted tensor; no
collectives.

v4 schedule (v2 history in kernel_v2_backup.py docstring). All deltas
HW-A/B-measured on TRN2 (noise +-15us, interleaved loop-delta):
  - K=128 score matmuls via zero-padded per-head stationaries (KTZ0/
    KTZ1, other head's 64 rows memset 0 once): K=64 mms measured
    ~2.5x slower than K=128 on HW -- this change alone was -110us.
  - eab (exp of scores) is fp8e4m3; att matmuls are fp8 DoubleRow
    over kb-pairs with STRIDE-80 stationary pairs (pair (p,h)
    contiguous at VB col 160*(2p+h)); a stride-160 pair layout makes
    DR a net loss, stride-80 beats 2 plain mms by ~25us.
  - P1 computes QKV only; qt0's scores+exp stream through a 2-deep
    scab ping-pong (4 PSUM banks, freed by deferring qt0's att); exps
    persist in 32 fp8 q0eab tiles (32KB SBUF).  qt0's att runs as a
    DR burst during qt1 bi0-7 into accumulators carved from the mlp
    PSUM ring (mlp mms only start at qt2), normalize at qt1 bi8.
  - All hot DMA layouts host-prearranged to single contiguous
    descriptors (k=8-strided APs cost 8x625ns HWDGE descriptors);
    fat P3 constants (w1/xres/prm) deferred to P2-start so they
    don't starve P1's x loads.
  - mlp matmuls emitted at +MLP_PRIO priority so the Tile scheduler
    runs them only when the score/att stream has nothing ready (-20us).
  - Cross-qtile software pipeline (carry) as v2: a qtile's last att
    batch + softmax normalize are deferred past the next qtile's first
    scores+exp.
  - LN via DVE bit-trick rsqrt, 1 Newton iter (no ACT table thrash);
    P3 row-blocks interleave into the exp stream as column-half-
    pipelined chunks.  reciprocal_approx_fast needs a base-partition-0
    input tile (custom DVE uops misread partition offsets).
Rejected by measurement: DVE i8-Schraudolph exp offload (neutral in
3 slot patterns), NR_ITERS=2 (neutral), exp batches of N=2048 (4-bank
PSUM reads ~3x slower per col), bf16 score PSUM (bass requires fp32
matmul output).
"""
import sys

if "/opt/trn_rl_repo" not in sys.path:
    sys.path.insert(0, "/opt/trn_rl_repo")

import numpy as np
from contextlib import ExitStack

import concourse.bacc as bacc
import concourse.mybir as mybir
import concourse.tile as tile
from concourse import bass2jax

f32 = mybir.dt.float32
f32r = mybir.dt.float32r
i32 = mybir.dt.int32
i16 = mybir.dt.int16
i8 = mybir.dt.int8
bf16 = mybir.dt.bfloat16
f8 = mybir.dt.float8e4
DR = mybir.MatmulPerfMode.DoubleRow
WS = 64.0             # host pre-scale on wq/wk/wv (fp8 subnormal escape)
Exp = mybir.ActivationFunctionType.Exp
Alu = mybir.AluOpType

N, D = 4096, 1024
_idx = np.arange(512)
_PERM = (_idx % 128 // 64) * 256 + (_idx % 64) * 4 + _idx // 128
EPS = 1e-5
ROWS = 512            # rows of the permuted tensor per core
NT = 8                # 512-wide tiles
KB = 32               # kpos blocks of 128 per q-tile
MAGIC = 0x5F3759DF    # rsqrt seed constant
# i8 Schraudolph: fp8e4m3 bits of ~exp(0.125*s) = int8(A8P*s + B8P)
A8P = 8 * 0.125 * 1.4426950
B8P = 8 * (7 - 0.0586)
VBW = 80              # VB block stride (65 used + 15 pad, 160B DR stride)
# (bi, h) chunks whose exp runs on DVE instead of ACT, for qt >= 2
DVE_BIS = ()
NR_ITERS = 1          # Newton iterations in the bit-trick rsqrt (~0.17% max err)
MLP_PRIO = 800        # priority offset pushing mlp mms behind the att stream
PLAIN_ATT = False     # True: 2 plain fp8 att mms per chunk instead of 1 DR mm
SCORE_K128 = True     # zero-padded stationary: K=128 score mms (fast PE path)
SCAB_BF16 = False     # bass requires fp32 matmul output; bf16 scab impossible
SCAB_BUFS = 1


def build(loop=0, phases=3, timing_reps=0, internal=False, dve_bis=None):
    """Build the per-core SPMD program. loop>0 wraps the body in For_i
    (timing variant)."""
    if dve_bis is None:
        dve_bis = DVE_BIS
    nc = bacc.Bacc("TRN2", target_bir_lowering=False, debug=False, num_devices=8)

    kind = "Internal" if (timing_reps or internal) else "ExternalInput"
    xTb_d = nc.dram_tensor("xTb", [128, 8 * N], f8, kind=kind).ap()
    wqT_d = nc.dram_tensor("wqT", [128, D], f8, kind=kind).ap()
    wkT_d = nc.dram_tensor("wkT", [128, D], f8, kind=kind).ap()
    wvT_d = nc.dram_tensor("wvT", [128, D], f8, kind=kind).ap()
    w1T_d = nc.dram_tensor("w1T", [D, D], bf16, kind=kind).ap()
    xres_d = nc.dram_tensor("xres", [ROWS, D], f32, kind=kind).ap()
    prm_d = nc.dram_tensor("prm", [128, 5 * D], f32, kind=kind).ap()
    ones_d = nc.dram_tensor("ones64", [1, 64], f32r, kind=kind).ap()
    idbf_d = nc.dram_tensor("idbf", [128, 128], bf16, kind=kind).ap()
    if timing_reps or internal:
        out_d = nc.dram_tensor("out", [ROWS, D], f32, kind="Internal").ap()
        tick_d = nc.dram_tensor("tick", [1, 4], f32, kind="ExternalOutput").ap()
    else:
        out_d = nc.dram_tensor("out", [ROWS, D], f32, kind="ExternalOutput").ap()
        tick_d = None

    with tile.TileContext(nc) as tc:
        with ExitStack() as ctx:
            const = ctx.enter_context(tc.tile_pool(name="const", bufs=1))
            main = ctx.enter_context(tc.tile_pool(name="main", bufs=1))

            # startup-critical constants on the SP queue, one DMA each
            # weights are host-prearranged to [128, k*128] so each load is a
            # single contiguous descriptor
            w_all = {}
            for pname, dram in (("wq", wqT_d), ("wk", wkT_d), ("wv", wvT_d)):
                t = const.tile([128, 8 * 128], f8, name=f"{pname}all")
                nc.sync.dma_start(t[:], dram[:])
                w_all[pname] = t
            idbf_t = const.tile([128, 128], bf16)
            nc.sync.dma_start(idbf_t[:], idbf_d[:])
            ones_t = const.tile([1, 64], f32r)
            nc.sync.dma_start(ones_t[:], ones_d[:])
            # P3-only constants: tiles here, DMA deferred to P2 start so the
            # transfers don't starve P1's xc loads (see load_p3_consts)
            w1_all = const.tile([128, 8 * D], bf16)
            xres_all = const.tile([128, 4 * D], f32)
            prm_all = const.tile([128, 5 * D], f32)

            def load_p3_consts():
                nc.scalar.dma_start(xres_all[:].rearrange("p (r o) -> p r o", r=4),
                                    xres_d.rearrange("(r p) o -> p r o", r=4))
                nc.scalar.dma_start(prm_all[:], prm_d[:])
                nc.scalar.dma_start(w1_all[:].rearrange("p (k o) -> p k o", k=8),
                                    w1T_d.rearrange("(k p) o -> p k o", k=8))
            # prm slices: b1b, g1b, bb1, g2b, bb2
            P_B1, P_G1, P_BB1, P_G2, P_BB2 = (
                prm_all[:, D * i:D * (i + 1)] for i in range(5))

            # persistent working tensors
            QT = main.tile([128, N], bf16)        # [2-head out dims, n]
            if SCORE_K128:
                # per-head stationary with the other head's rows hard-zeroed:
                # K=128 score mms (full-partition operands run ~2.5x faster
                # than K=64 on HW); zeros written once, never touched again
                KT = [main.tile([128, N], bf16, name=f"KTZ{h}") for h in range(2)]
                nc.vector.memset(KT[0][64:128, :], 0.0)
                nc.vector.memset(KT[1][0:64, :], 0.0)
            else:
                KT0 = main.tile([128, N], bf16)
                KT = [KT0, KT0]
            # VB: fp8 V^T+ones; kb-pair p head h contiguous at col 160*(2p+h)
            # (kb=2p at +0, kb=2p+1 at +80) so the DR stationary stride is 80
            VB = main.tile([128, 2 * VBW * 32], f8)
            nc.vector.memset(VB[:], 1.0)
            att_perm = [main.tile([128, D], f32, name=f"attperm{r}") for r in range(4)]

            def body(_=None):
                with ExitStack() as cb:
                    p2sb = cb.enter_context(tc.tile_pool(name="p2sb", bufs=1))
                    p3sb = cb.enter_context(tc.tile_pool(name="p3sb", bufs=1))
                    q0e = cb.enter_context(tc.tile_pool(name="q0e", bufs=1))
                    # qt0's exps, persisted until the qt1 att burst
                    q0eab = [q0e.tile([128, 1024], f8, name=f"q0eab{c}")
                             for c in range(KB)]

                    # ---------- shared helpers ----------
                    def rsqrt_dve(y, ve, t, u):
                        """y = 1/sqrt(ve), all [128,1] f32; t/u scratch."""
                        nc.vector.tensor_scalar(y[:].bitcast(i32), ve[:].bitcast(i32),
                                                1, None, op0=Alu.arith_shift_right)
                        nc.vector.tensor_scalar(y[:].bitcast(i32), y[:].bitcast(i32),
                                                -1, None, op0=Alu.bitwise_xor)
                        nc.vector.tensor_scalar(y[:].bitcast(i32), y[:].bitcast(i32),
                                                MAGIC + 1, None, op0=Alu.add)
                        for _i in range(NR_ITERS):
                            nc.vector.tensor_mul(t[:], y[:], y[:])
                            nc.vector.tensor_mul(t[:], t[:], ve[:])
                            nc.vector.tensor_scalar(u[:], t[:], -0.5, 1.5,
                                                    op0=Alu.mult, op1=Alu.add)
                            nc.vector.tensor_mul(y[:], y[:], u[:])

                    def att_dr(dst, h, p, eab_t, start, stop):
                        """att mms for kb-pair p, head h, into dst [65,512]."""
                        b0 = 2 * VBW * (2 * p + h)
                        if PLAIN_ATT:
                            for a in range(2):
                                nc.tensor.matmul(
                                    dst, VB[:, b0 + VBW * a:b0 + VBW * a + 65],
                                    eab_t[:, 512 * a:512 * (a + 1)],
                                    start=start and a == 0, stop=stop and a == 1)
                        else:
                            nc.tensor.matmul(
                                dst,
                                VB[:, b0:b0 + 2 * VBW]
                                    .rearrange("p (a b) -> p a b", a=2)[:, :, 0:65],
                                eab_t[:].rearrange("p (a b) -> p a b", a=2),
                                start=start, stop=stop, perf_mode=DR)

                    def score_mm(dst, h, kb, qsl):
                        if SCORE_K128:
                            nc.tensor.matmul(
                                dst, KT[h][:, 128 * kb:128 * (kb + 1)],
                                QT[:, qsl], start=True, stop=True)
                        else:
                            nc.tensor.matmul(
                                dst, KT[h][64 * h:64 * (h + 1), 128 * kb:128 * (kb + 1)],
                                QT[64 * h:64 * (h + 1), qsl],
                                start=True, stop=True)

                    def normalize(accs, n_hi, c0):
                        """softmax-normalize att psum accs and write att_perm.
                        accs[h]: tile with [0:65, :512] holding att+rowsum."""
                        for h in range(2):
                            acc = p2sb.tile([65, 512], f32, tag=f"acc{h}", bufs=2)
                            nc.vector.tensor_copy(acc[:], accs[h][0:65, :])
                            rsum = p2sb.tile([1, 512], f32, tag="rsum", bufs=2)
                            nc.vector.tensor_copy(rsum[:], acc[64:65, :])
                            rec = p2sb.tile([1, 512], f32, tag="rec", bufs=2)
                            # reciprocal_approx (custom DVE uop) misreads
                            # partition-offset inputs -- rsum must be base 0
                            nc.vector.reciprocal_approx_fast(rec[:], rsum[:])
                            rec_r = p2sb.tile([1, 512], f32r, tag="recr", bufs=2)
                            nc.vector.tensor_copy(rec_r[:], rec[:].bitcast(f32r))
                            # K=1 partition-broadcast matmul into the drained bank
                            nc.tensor.matmul(accs[h][0:64, :], ones_t[:], rec_r[:],
                                             start=True, stop=True)
                            nc.vector.tensor_mul(
                                att_perm[n_hi][64 * h:64 * (h + 1), c0:c0 + 512],
                                acc[0:64, :], accs[h][0:64, :])

                    # ================= P1: QKV + qt0 scores/exp ================
                    with ExitStack() as c1:
                        p1sb = c1.enter_context(tc.tile_pool(name="p1sb", bufs=1))
                        p1ps = c1.enter_context(tc.tile_pool(name="p1ps", bufs=1, space="PSUM"))
                        trps = c1.enter_context(tc.tile_pool(name="trps", bufs=1, space="PSUM"))
                        q0ps = c1.enter_context(tc.tile_pool(name="q0ps", bufs=1, space="PSUM"))

                        def q0_chunk(p, h):
                            """scores+exp for qt0 kb-pair p, head h."""
                            scab = q0ps.tile([128, 1024], bf16 if SCAB_BF16 else f32,
                                             tag=f"q0s{h}",
                                             bufs=SCAB_BUFS if SCAB_BF16 else 1)
                            for j, kb in enumerate((2 * p, 2 * p + 1)):
                                score_mm(scab[:, 512 * j:512 * (j + 1)], h, kb,
                                         slice(0, 512))
                            nc.scalar.activation(q0eab[2 * p + h][:], scab[:],
                                                 Exp, scale=0.125)

                        def load_xc(nt):
                            # host layout is nt-major: one contiguous descriptor
                            xc = p1sb.tile([128, 8 * 512], f8, tag="xc", bufs=2)
                            nc.sync.dma_start(xc[:],
                                              xTb_d[:, 4096 * nt:4096 * (nt + 1)])
                            return xc

                        xc_next = load_xc(0)
                        for nt in range(NT):
                            xc = xc_next
                            if nt + 1 < NT:
                                xc_next = load_xc(nt + 1)
                            chunks = ([(p, h) for p in (2 * nt - 2, 2 * nt - 1)
                                       for h in (0, 1)] if nt > 0 else [])
                            ps = {}
                            for gi, pname in enumerate(("wq", "wk", "wv")):
                                if gi < len(chunks):
                                    q0_chunk(*chunks[gi])
                                p_t = p1ps.tile([128, 512], f32, tag=f"{pname}ps")
                                for k2 in range(4):
                                    nc.tensor.matmul(
                                        p_t[:],
                                        w_all[pname][:, 256 * k2:256 * (k2 + 1)]
                                            .rearrange("p (i o) -> p i o", i=2),
                                        xc[:, 1024 * k2:1024 * (k2 + 1)]
                                            .rearrange("p (i n) -> p i n", i=2),
                                        start=(k2 == 0), stop=(k2 == 3), perf_mode=DR)
                                ps[pname] = p_t
                            sl = slice(512 * nt, 512 * (nt + 1))
                            nc.vector.tensor_scalar_mul(QT[:, sl], ps["wq"][:], 1.0 / WS)
                            if SCORE_K128:
                                nc.vector.tensor_scalar_mul(
                                    KT[0][0:64, sl], ps["wk"][0:64, :], 1.0 / WS)
                                nc.vector.tensor_scalar_mul(
                                    KT[1][64:128, sl], ps["wk"][64:128, :], 1.0 / WS)
                            else:
                                nc.vector.tensor_scalar_mul(KT[0][:, sl], ps["wk"][:],
                                                            1.0 / WS)
                            VT = p1sb.tile([128, 512], bf16, tag="VT", bufs=2)
                            nc.vector.tensor_scalar_mul(VT[:], ps["wv"][:], 1.0 / WS)
                            tp = trps.tile([128, 512], bf16, tag="tp")
                            for b in range(4):
                                nc.tensor.transpose(
                                    tp[:, 128 * b:128 * (b + 1)],
                                    VT[:, 128 * b:128 * (b + 1)], idbf_t[:])
                            # tp block b (kb=4nt+b) cols [64h:64h+64] -> VB col
                            # 160*(2*(kb//2)+h) + 80*(kb%2); per h two 2-block
                            # strided copies (pairs (0,1) and (2,3))
                            for h in range(2):
                                for g in range(2):
                                    base = 2 * VBW * (4 * nt + 2 * g + h)
                                    nc.vector.tensor_copy(
                                        VB[:, base:base + 2 * VBW]
                                          .rearrange("p (b e) -> p b e", b=2)[:, :, 0:64],
                                        tp[:, 256 * g:256 * (g + 1)]
                                          .rearrange("p (b e) -> p b e", b=2)[:, :, 64 * h:64 * h + 64])
                            if len(chunks) > 3:
                                q0_chunk(*chunks[3])
                        for p in (2 * NT - 2, 2 * NT - 1):
                            for h in range(2):
                                q0_chunk(p, h)

                    if phases == 1:   # debug: dump QT/KT/VB + q0eab[0:4]
                        nc.sync.dma_start(out_d[0:128, :], QT[:, 0:2048].bitcast(f32))
                        nc.sync.dma_start(out_d[128:256, :], KT[0][:, 0:2048].bitcast(f32))
                        nc.sync.dma_start(out_d[256:384, :], VB[:, 0:4096].bitcast(f32))
                        for j in range(4):
                            nc.sync.dma_start(out_d[384:512, 256 * j:256 * (j + 1)],
                                              q0eab[j][:].bitcast(f32))
                        return

                    # ================= P2: qt1..7 + qt0 burst + P3 ================
                    with ExitStack() as c2:
                        if phases >= 3:
                            load_p3_consts()
                        atps = c2.enter_context(tc.tile_pool(name="atps", bufs=1, space="PSUM"))
                        scps = c2.enter_context(tc.tile_pool(name="scps", bufs=1, space="PSUM"))
                        mlps = c2.enter_context(tc.tile_pool(name="mlps", bufs=2, space="PSUM"))

                        attps0 = [atps.tile([65, 512], f32, name=f"attps{h}")
                                  for h in range(2)]
                        # qt0 att accumulators, carved from the mlp ring (mlp
                        # mms only start at qt2)
                        q0acc = [mlps.tile([128, 512], f32, tag="mp",
                                           name=f"q0acc{h}") for h in range(2)]

                        def ln_stats(state, key):
                            """aggr + bit-trick rsqrt from state[key+'st6']."""
                            ag = p3sb.tile([128, 2], f32, tag="ag", bufs=2)
                            nc.vector.bn_aggr(ag[:], state[key + "st6"][:])
                            nmu = p3sb.tile([128, 1], f32, tag="nmu", bufs=2)
                            nc.vector.tensor_scalar_mul(nmu[:], ag[:, 0:1], -1.0)
                            ve = p3sb.tile([128, 1], f32, tag="ve", bufs=2)
                            nc.vector.tensor_scalar_add(ve[:], ag[:, 1:2], EPS)
                            inv = p3sb.tile([128, 1], f32, tag="inv", bufs=2)
                            ts = p3sb.tile([128, 1], f32, tag="ts", bufs=2)
                            us = p3sb.tile([128, 1], f32, tag="us", bufs=2)
                            rsqrt_dve(inv, ve, ts, us)
                            state[key + "nmu"], state[key + "inv"] = nmu, inv

                        def p3_block(r):
                            """P3 for row block r: (head_chunk, rest_chunks)."""
                            state = {}
                            xres_r = xres_all[:, D * r:D * (r + 1)]

                            def c_s1a():   # runs one qtile early (half 0 only)
                                s1 = p3sb.tile([128, D], f32, tag="s1", bufs=2)
                                st6 = p3sb.tile([128, 2, 6], f32, tag="st6a", bufs=2)
                                nc.vector.tensor_add(s1[:, 0:512], att_perm[r][:, 0:512],
                                                     xres_r[:, 0:512])
                                nc.vector.bn_stats(st6[:, 0, :], s1[:, 0:512])
                                state["s1"], state["ast6"] = s1, st6

                            def c_s1b():
                                s1 = state["s1"]
                                nc.vector.tensor_add(s1[:, 512:1024],
                                                     att_perm[r][:, 512:1024],
                                                     xres_r[:, 512:1024])
                                nc.vector.bn_stats(state["ast6"][:, 1, :], s1[:, 512:1024])
                                ln_stats(state, "a")
                                state["h1"] = p3sb.tile([128, D], bf16, tag="h1", bufs=2,
                                                        name="h1")
                                state["h1T"] = p3sb.tile([128, D], bf16, tag="h1T", bufs=2,
                                                         name="h1T")
                                state["s2"] = p3sb.tile([128, D], f32, tag="s2", bufs=2,
                                                        name="s2")
                                state["bst6"] = p3sb.tile([128, 2, 6], f32, tag="st6b",
                                                          bufs=2, name="bst6")

                            def c_nh(j):   # LN1 apply + transpose, half j
                                s1, h1 = state["s1"], state["h1"]
                                sl = slice(512 * j, 512 * (j + 1))
                                nc.vector.scalar_tensor_tensor(
                                    s1[:, sl], s1[:, sl], state["anmu"], P_G1[:, sl],
                                    op0=Alu.add, op1=Alu.mult)
                                nc.vector.scalar_tensor_tensor(
                                    h1[:, sl], s1[:, sl], state["ainv"], P_BB1[:, sl],
                                    op0=Alu.mult, op1=Alu.add)
                                nc.sync.dma_start(
                                    state["h1T"][:, sl].rearrange("p (cb r2) -> p cb r2", cb=4),
                                    h1[:, sl], transpose=True)

                            def c_mlp(jt, half):   # 4 matmuls; finishes s2 half jt
                                jsl = slice(512 * jt, 512 * (jt + 1))
                                if half == 0:
                                    state[f"mp{jt}"] = mlps.tile([128, 512], f32, tag="mp",
                                                                 name=f"mp{jt}")
                                mp = state[f"mp{jt}"]
                                # deprioritized: PE runs these only when the
                                # score/att stream has nothing ready
                                prio0 = tc.cur_priority
                                tc.cur_priority += MLP_PRIO
                                for cb in range(4 * half, 4 * half + 4):
                                    nc.tensor.matmul(
                                        mp[:], state["h1T"][:, 128 * cb:128 * (cb + 1)],
                                        w1_all[:, D * cb + 512 * jt:D * cb + 512 * (jt + 1)],
                                        start=(cb == 0), stop=(cb == 7))
                                tc.cur_priority = prio0
                                if half == 1:
                                    s2 = state["s2"]
                                    nc.vector.scalar_tensor_tensor(
                                        s2[:, jsl], mp[:], 1.0, P_B1[:, jsl],
                                        op0=Alu.mult, op1=Alu.add)
                                    nc.vector.tensor_add(s2[:, jsl], s2[:, jsl],
                                                         state["h1"][:, jsl])
                                    nc.vector.bn_stats(state["bst6"][:, jt, :], s2[:, jsl])

                            def c_f(j):   # LN2 apply + store, half j
                                if j == 0:
                                    ln_stats(state, "b")
                                    state["ot"] = p3sb.tile([128, D], f32, tag="ot",
                                                            bufs=2, name="ot")
                                s2, o_t = state["s2"], state["ot"]
                                sl = slice(512 * j, 512 * (j + 1))
                                nc.vector.scalar_tensor_tensor(
                                    s2[:, sl], s2[:, sl], state["bnmu"], P_G2[:, sl],
                                    op0=Alu.add, op1=Alu.mult)
                                nc.vector.scalar_tensor_tensor(
                                    o_t[:, sl], s2[:, sl], state["binv"], P_BB2[:, sl],
                                    op0=Alu.mult, op1=Alu.add)
                                nc.sync.dma_start(out_d[128 * r:128 * (r + 1), sl],
                                                  o_t[:, sl])

                            return c_s1a, [c_s1b, lambda: c_nh(0), lambda: c_nh(1),
                                           lambda: c_mlp(0, 0), lambda: c_mlp(0, 1),
                                           lambda: c_mlp(1, 0), lambda: c_mlp(1, 1),
                                           lambda: c_f(0), lambda: c_f(1)]

                        p3_blocks = [p3_block(r) for r in range(4)] if phases >= 3 else None
                        p3_queue = []
                        carry = [None]

                        def run_carry():
                            if carry[0] is not None:
                                carry[0]()
                                carry[0] = None

                        for qt in range(1, NT):
                            qsl = slice(512 * qt, 512 * (qt + 1))
                            if phases >= 3 and qt % 2 == 1:
                                p3_queue.append(p3_blocks[(qt - 1) // 2][0])
                            pend = []
                            for bi in range(16):
                                eabs = []
                                for h in range(2):
                                    scab = scps.tile([128, 1024],
                                                     bf16 if SCAB_BF16 else f32,
                                                     tag=f"scab{h}", name=f"scab{h}",
                                                     bufs=SCAB_BUFS if SCAB_BF16 else 1)
                                    for j, kb in enumerate((2 * bi, 2 * bi + 1)):
                                        score_mm(scab[:, 512 * j:512 * (j + 1)],
                                                 h, kb, qsl)
                                    eab = p2sb.tile([128, 1024], f8, tag=f"eab{h}",
                                                    name=f"eab{h}", bufs=6)
                                    if qt >= 2 and (bi, h) in dve_bis:
                                        nc.vector.tensor_scalar(
                                            eab[:].bitcast(i8), scab[:], A8P, B8P,
                                            op0=Alu.mult, op1=Alu.add)
                                    else:
                                        nc.scalar.activation(eab[:], scab[:], Exp,
                                                             scale=0.125)
                                    eabs.append(eab)
                                if bi == 0:
                                    run_carry()   # prev qtile's last att + normalize
                                # atts trail exps by 2 bi so the carry's
                                # normalize gets 2 bi to release attps; at each
                                # qtile's end pop TWO per bi so the carry holds
                                # only one att batch -> shorter carry, attps
                                # released earlier at every qtile boundary
                                pops = 2 if bi >= 14 else 1
                                for _ in range(pops):
                                    if len(pend) >= (2 if pops == 1 else 1):
                                        pb, peabs = pend.pop(0)
                                        for h in range(2):
                                            att_dr(attps0[h][0:65, :], h, pb,
                                                   peabs[h], start=(pb == 0),
                                                   stop=(pb == 15))
                                pend.append((bi, eabs))
                                if qt == 1:
                                    if bi < 8:   # qt0 att burst: pairs 2bi, 2bi+1
                                        # deprioritized: fills PE gaps in qt1's
                                        # own stream; must finish by bi8
                                        prio0 = tc.cur_priority
                                        tc.cur_priority += 300
                                        for p in (2 * bi, 2 * bi + 1):
                                            for h in range(2):
                                                att_dr(q0acc[h][0:65, :], h, p,
                                                       q0eab[2 * p + h],
                                                       start=(p == 0), stop=(p == 15))
                                        tc.cur_priority = prio0
                                    elif bi == 8:
                                        normalize(q0acc, 0, 0)
                                    elif bi == 11 and phases >= 3 and p3_queue:
                                        p3_queue.pop(0)()
                                elif p3_queue and bi in (2, 4, 6, 8, 10, 12):
                                    p3_queue.pop(0)()

                            def qt_tail(pend=pend, qt=qt):
                                for pb, peabs in pend:
                                    for h in range(2):
                                        att_dr(attps0[h][0:65, :], h, pb, peabs[h],
                                               start=(pb == 0), stop=(pb == 15))
                                normalize(attps0, qt // 2, 512 * (qt % 2))
                            carry[0] = qt_tail
                            if phases >= 3 and qt % 2 == 1:
                                p3_queue.extend(p3_blocks[(qt - 1) // 2][1])
                        run_carry()
                        if phases == 2:   # debug: dump att_perm
                            for r in range(4):
                                nc.sync.dma_start(out_d[128 * r:128 * (r + 1), :],
                                                  att_perm[r][:])
                            return
                        # drain remaining P3 work (r=3)
                        while p3_queue:
                            p3_queue.pop(0)()

            if timing_reps:
                for _rep in range(timing_reps):
                    body()
            elif loop:
                with tc.For_i(0, loop, 1, staggered_reset=True,
                              hint_engines=mybir.ALL_ENGINES) as _:
                    body()
            else:
                body()
            if tick_d is not None:
                tick = main.tile([1, 4], f32)
                nc.vector.tensor_copy(tick[:], QT[0:1, 0:8].bitcast(f32))
                nc.sync.dma_start(tick_d[:], tick[:])
    nc.compile()
    return nc


_CACHE = {}


def _get_nc(loop=0, phases=3, timing_reps=0, internal=False, dve_bis=None):
    key = (loop, phases, timing_reps, internal, dve_bis)
    if key not in _CACHE:
        _CACHE[key] = build(loop, phases, timing_reps, internal, dve_bis)
    return _CACHE[key]


def make_in_maps(x, wq, wk, wv, ln1_g, ln1_b, w1, b1, ln2_g, ln2_b):
    import ml_dtypes
    x = np.asarray(x, np.float32)
    xT = np.ascontiguousarray(x.T)  # [D, N]
    # nt-major: xTb[p, 4096*nt + 512*k + c] = xT[128k+p, 512nt+c], so each
    # per-nt xc load is one contiguous [128, 4096] descriptor
    xTb = np.ascontiguousarray(
        xT.reshape(8, 128, 8, 512).transpose(1, 2, 0, 3).reshape(128, 8 * N)
    ).astype(ml_dtypes.float8_e4m3)

    def w_pre(w, rs):
        wT = np.ascontiguousarray(np.asarray(w, np.float32)[rs].T * 64.0)  # [D,128]
        return np.ascontiguousarray(
            wT.reshape(8, 128, 128).transpose(1, 0, 2).reshape(128, D)
        ).astype(ml_dtypes.float8_e4m3)
    w1T = np.ascontiguousarray(np.asarray(w1, np.float32).T).astype(ml_dtypes.bfloat16)
    bcast = lambda v: np.broadcast_to(np.asarray(v, np.float32), (128, D))
    prm = np.ascontiguousarray(np.concatenate(
        [bcast(b1), bcast(ln1_g), bcast(ln1_b), bcast(ln2_g), bcast(ln2_b)], axis=1))
    ones64 = np.ones((1, 64), np.float32)
    idbf = np.eye(128, dtype=ml_dtypes.bfloat16)
    in_maps = []
    for c in range(8):
        rs = slice(128 * c, 128 * (c + 1))
        in_maps.append({
            "xTb": xTb,
            "wqT": w_pre(wq, rs),
            "wkT": w_pre(wk, rs),
            "wvT": w_pre(wv, rs),
            "w1T": w1T,
            "xres": np.ascontiguousarray(x[512 * c:512 * (c + 1)][_PERM]),
            "prm": prm,
            "ones64": ones64, "idbf": idbf,
        })
    return in_maps


def kernel(**inputs):
    nc = _get_nc(0)
    in_maps = make_in_maps(**inputs)
    results = bass2jax.run_bass_via_pjrt(nc, in_maps, n_cores=8)
    outs = []
    for c in range(8):
        o = np.empty((ROWS, D), np.float32)
        o[_PERM] = results[c]["out"]
        outs.append(o)
    return np.concatenate(outs, axis=0).astype(np.float32)


# revision 46
# speedup vs baseline: 1.2743x; 1.2743x over previous
"""Trainium2 Bass kernel for nn_Attention_37847251812733.

Full transformer block: QKV proj -> 16-head attention (N=4096, DH=64)
-> permuted reshape (the reference's transpose(1,2).reshape) -> LN ->
MLP -> LN.  Tensor-parallel over heads; core c owns heads {2c, 2c+1}
and produces rows [512c, 512c+512) of the permuted tensor; no
collectives.

v4 schedule (v2 history in kernel_v2_backup.py docstring). All deltas
HW-A/B-measured on TRN2 (noise +-15us, interleaved loop-delta):
  - K=128 score matmuls via zero-padded per-head stationaries (KTZ0/
    KTZ1, other head's 64 rows memset 0 once): K=64 mms measured
    ~2.5x slower than K=128 on HW -- this change alone was -110us.
  - eab (exp of scores) is fp8e4m3; att matmuls are fp8 DoubleRow
    over kb-pairs with STRIDE-80 stationary pairs (pair (p,h)
    contiguous at VB col 160*(2p+h)); a stride-160 pair layout makes
    DR a net loss, stride-80 beats 2 plain mms by ~25us.
  - P1 computes QKV only; qt0's scores+exp stream through a 2-deep
    scab ping-pong (4 PSUM banks, freed by deferring qt0's att); exps
    persist in 32 fp8 q0eab tiles (32KB SBUF).  qt0's att runs as a
    DR burst during qt1 bi0-7 into accumulators carved from the mlp
    PSUM ring (mlp mms only start at qt2), normalize at qt1 bi8.
  - All hot DMA layouts host-prearranged to single contiguous
    descriptors (k=8-strided APs cost 8x625ns HWDGE descriptors);
    fat P3 constants (w1/xres/prm) deferred to P2-start so they
    don't starve P1's x loads.
  - mlp matmuls emitted at +MLP_PRIO priority so the Tile scheduler
    runs them only when the score/att stream has nothing ready (-20us).
  - Cross-qtile software pipeline (carry) as v2: a qtile's last att
    batch + softmax normalize are deferred past the next qtile's first
    scores+exp.
  - LN via DVE bit-trick rsqrt, 1 Newton iter (no ACT table thrash);
    P3 row-blocks interleave into the exp stream as column-half-
    pipelined chunks.  reciprocal_approx_fast needs a base-partition-0
    input tile (custom DVE uops misread partition offsets).
Rejected by measurement: DVE i8-Schraudolph exp offload (neutral in
3 slot patterns), NR_ITERS=2 (neutral), exp batches of N=2048 (4-bank
PSUM reads ~3x slower per col), bf16 score PSUM (bass requires fp32
matmul output).
"""
import sys

if "/opt/trn_rl_repo" not in sys.path:
    sys.path.insert(0, "/opt/trn_rl_repo")

import numpy as np
from contextlib import ExitStack

import concourse.bacc as bacc
import concourse.mybir as mybir
import concourse.tile as tile
from concourse import bass2jax

f32 = mybir.dt.float32
f32r = mybir.dt.float32r
i32 = mybir.dt.int32
i16 = mybir.dt.int16
i8 = mybir.dt.int8
bf16 = mybir.dt.bfloat16
f8 = mybir.dt.float8e4
DR = mybir.MatmulPerfMode.DoubleRow
WS = 64.0             # host pre-scale on wq/wk/wv (fp8 subnormal escape)
Exp = mybir.ActivationFunctionType.Exp
Alu = mybir.AluOpType

N, D = 4096, 1024
_idx = np.arange(512)
_PERM = (_idx % 128 // 64) * 256 + (_idx % 64) * 4 + _idx // 128
EPS = 1e-5
ROWS = 512            # rows of the permuted tensor per core
NT = 8                # 512-wide tiles
KB = 32               # kpos blocks of 128 per q-tile
MAGIC = 0x5F3759DF    # rsqrt seed constant
# i8 Schraudolph: fp8e4m3 bits of ~exp(0.125*s) = int8(A8P*s + B8P)
A8P = 8 * 0.125 * 1.4426950
B8P = 8 * (7 - 0.0586)
VBW = 80              # VB block stride (65 used + 15 pad, 160B DR stride)
# (bi, h) chunks whose exp runs on DVE instead of ACT, for qt >= 2
DVE_BIS = ()
NR_ITERS = 1          # Newton iterations in the bit-trick rsqrt (~0.17% max err)
MLP_PRIO = 800        # priority offset pushing mlp mms behind the att stream
PLAIN_ATT = False     # True: 2 plain fp8 att mms per chunk instead of 1 DR mm
SCORE_K128 = True     # zero-padded stationary: K=128 score mms (fast PE path)
SCAB_BF16 = False     # bass requires fp32 matmul output; bf16 scab impossible
SCAB_BUFS = 1


def build(loop=0, phases=3, timing_reps=0, internal=False, dve_bis=None):
    """Build the per-core SPMD program. loop>0 wraps the body in For_i
    (timing variant)."""
    if dve_bis is None:
        dve_bis = DVE_BIS
    nc = bacc.Bacc("TRN2", target_bir_lowering=False, debug=False, num_devices=8)

    kind = "Internal" if (timing_reps or internal) else "ExternalInput"
    xTb_d = nc.dram_tensor("xTb", [128, 8 * N], f8, kind=kind).ap()
    wqT_d = nc.dram_tensor("wqT", [128, D], f8, kind=kind).ap()
    wkT_d = nc.dram_tensor("wkT", [128, D], f8, kind=kind).ap()
    wvT_d = nc.dram_tensor("wvT", [128, D], f8, kind=kind).ap()
    w1T_d = nc.dram_tensor("w1T", [D, D], bf16, kind=kind).ap()
    xres_d = nc.dram_tensor("xres", [ROWS, D], f32, kind=kind).ap()
    prm_d = nc.dram_tensor("prm", [128, 5 * D], f32, kind=kind).ap()
    ones_d = nc.dram_tensor("ones64", [1, 64], f32r, kind=kind).ap()
    idbf_d = nc.dram_tensor("idbf", [128, 128], bf16, kind=kind).ap()
    if timing_reps or internal:
        out_d = nc.dram_tensor("out", [ROWS, D], f32, kind="Internal").ap()
        tick_d = nc.dram_tensor("tick", [1, 4], f32, kind="ExternalOutput").ap()
    else:
        out_d = nc.dram_tensor("out", [ROWS, D], f32, kind="ExternalOutput").ap()
        tick_d = None

    with tile.TileContext(nc) as tc:
        with ExitStack() as ctx:
            const = ctx.enter_context(tc.tile_pool(name="const", bufs=1))
            main = ctx.enter_context(tc.tile_pool(name="main", bufs=1))

            # startup-critical constants on the SP queue, one DMA each
            # weights are host-prearranged to [128, k*128] so each load is a
            # single contiguous descriptor
            w_all = {}
            for pname, dram in (("wq", wqT_d), ("wk", wkT_d), ("wv", wvT_d)):
                t = const.tile([128, 8 * 128], f8, name=f"{pname}all")
                nc.sync.dma_start(t[:], dram[:])
                w_all[pname] = t
            idbf_t = const.tile([128, 128], bf16)
            nc.sync.dma_start(idbf_t[:], idbf_d[:])
            ones_t = const.tile([1, 64], f32r)
            nc.sync.dma_start(ones_t[:], ones_d[:])
            # P3-only constants: tiles here, DMA deferred to P2 start so the
            # transfers don't starve P1's xc loads (see load_p3_consts)
            w1_all = const.tile([128, 8 * D], bf16)
            xres_all = const.tile([128, 4 * D], f32)
            prm_all = const.tile([128, 5 * D], f32)

            def load_p3_consts():
                nc.scalar.dma_start(xres_all[:].rearrange("p (r o) -> p r o", r=4),
                                    xres_d.rearrange("(r p) o -> p r o", r=4))
                nc.scalar.dma_start(prm_all[:], prm_d[:])
                nc.scalar.dma_start(w1_all[:].rearrange("p (k o) -> p k o", k=8),
                                    w1T_d.rearrange("(k p) o -> p k o", k=8))
            # prm slices: b1b, g1b, bb1, g2b, bb2
            P_B1, P_G1, P_BB1, P_G2, P_BB2 = (
                prm_all[:, D * i:D * (i + 1)] for i in range(5))

            # persistent working tensors
            QT = main.tile([128, N], bf16)        # [2-head out dims, n]
            if SCORE_K128:
                # per-head stationary with the other head's rows hard-zeroed:
                # K=128 score mms (full-partition operands run ~2.5x faster
                # than K=64 on HW); zeros written once, never touched again
                KT = [main.tile([128, N], bf16, name=f"KTZ{h}") for h in range(2)]
                nc.vector.memset(KT[0][64:128, :], 0.0)
                nc.vector.memset(KT[1][0:64, :], 0.0)
            else:
                KT0 = main.tile([128, N], bf16)
                KT = [KT0, KT0]
            # VB: fp8 V^T+ones; kb-pair p head h contiguous at col 160*(2p+h)
            # (kb=2p at +0, kb=2p+1 at +80) so the DR stationary stride is 80
            VB = main.tile([128, 2 * VBW * 32], f8)
            nc.vector.memset(VB[:], 1.0)
            att_perm = [main.tile([128, D], f32, name=f"attperm{r}") for r in range(4)]

            def body(_=None):
                with ExitStack() as cb:
                    p2sb = cb.enter_context(tc.tile_pool(name="p2sb", bufs=1))
                    p3sb = cb.enter_context(tc.tile_pool(name="p3sb", bufs=1))
                    q0e = cb.enter_context(tc.tile_pool(name="q0e", bufs=1))
                    # qt0's exps, persisted until the qt1 att burst
                    q0eab = [q0e.tile([128, 1024], f8, name=f"q0eab{c}")
                             for c in range(KB)]

                    # ---------- shared helpers ----------
                    def rsqrt_dve(y, ve, t, u):
                        """y = 1/sqrt(ve), all [128,1] f32; t/u scratch."""
                        nc.vector.tensor_scalar(y[:].bitcast(i32), ve[:].bitcast(i32),
                                                1, None, op0=Alu.arith_shift_right)
                        nc.vector.tensor_scalar(y[:].bitcast(i32), y[:].bitcast(i32),
                                                -1, None, op0=Alu.bitwise_xor)
                        nc.vector.tensor_scalar(y[:].bitcast(i32), y[:].bitcast(i32),
                                                MAGIC + 1, None, op0=Alu.add)
                        for _i in range(NR_ITERS):
                            nc.vector.tensor_mul(t[:], y[:], y[:])
                            nc.vector.tensor_mul(t[:], t[:], ve[:])
                            nc.vector.tensor_scalar(u[:], t[:], -0.5, 1.5,
                                                    op0=Alu.mult, op1=Alu.add)
                            nc.vector.tensor_mul(y[:], y[:], u[:])

                    def att_dr(dst, h, p, eab_t, start, stop):
                        """att mms for kb-pair p, head h, into dst [65,512]."""
                        b0 = 2 * VBW * (2 * p + h)
                        if PLAIN_ATT:
                            for a in range(2):
                                nc.tensor.matmul(
                                    dst, VB[:, b0 + VBW * a:b0 + VBW * a + 65],
                                    eab_t[:, 512 * a:512 * (a + 1)],
                                    start=start and a == 0, stop=stop and a == 1)
                        else:
                            nc.tensor.matmul(
                                dst,
                                VB[:, b0:b0 + 2 * VBW]
                                    .rearrange("p (a b) -> p a b", a=2)[:, :, 0:65],
                                eab_t[:].rearrange("p (a b) -> p a b", a=2),
                                start=start, stop=stop, perf_mode=DR)

                    def score_mm(dst, h, kb, qsl):
                        if SCORE_K128:
                            nc.tensor.matmul(
                                dst, KT[h][:, 128 * kb:128 * (kb + 1)],
                                QT[:, qsl], start=True, stop=True)
                        else:
                            nc.tensor.matmul(
                                dst, KT[h][64 * h:64 * (h + 1), 128 * kb:128 * (kb + 1)],
                                QT[64 * h:64 * (h + 1), qsl],
                                start=True, stop=True)

                    def normalize(accs, n_hi, c0):
                        """softmax-normalize att psum accs and write att_perm.
                        accs[h]: tile with [0:65, :512] holding att+rowsum."""
                        for h in range(2):
                            acc = p2sb.tile([65, 512], f32, tag=f"acc{h}", bufs=2)
                            nc.vector.tensor_copy(acc[:], accs[h][0:65, :])
                            rsum = p2sb.tile([1, 512], f32, tag="rsum", bufs=2)
                            nc.vector.tensor_copy(rsum[:], acc[64:65, :])
                            rec = p2sb.tile([1, 512], f32, tag="rec", bufs=2)
                            # reciprocal_approx (custom DVE uop) misreads
                            # partition-offset inputs -- rsum must be base 0
                            nc.vector.reciprocal_approx_fast(rec[:], rsum[:])
                            rec_r = p2sb.tile([1, 512], f32r, tag="recr", bufs=2)
                            nc.vector.tensor_copy(rec_r[:], rec[:].bitcast(f32r))
                            # K=1 partition-broadcast matmul into the drained bank
                            nc.tensor.matmul(accs[h][0:64, :], ones_t[:], rec_r[:],
                                             start=True, stop=True)
                            nc.vector.tensor_mul(
                                att_perm[n_hi][64 * h:64 * (h + 1), c0:c0 + 512],
                                acc[0:64, :], accs[h][0:64, :])

                    # ================= P1: QKV + qt0 scores/exp ================
                    with ExitStack() as c1:
                        p1sb = c1.enter_context(tc.tile_pool(name="p1sb", bufs=1))
                        p1ps = c1.enter_context(tc.tile_pool(name="p1ps", bufs=1, space="PSUM"))
                        trps = c1.enter_context(tc.tile_pool(name="trps", bufs=1, space="PSUM"))
                        q0ps = c1.enter_context(tc.tile_pool(name="q0ps", bufs=1, space="PSUM"))

                        def q0_chunk(p, h):
                            """scores+exp for qt0 kb-pair p, head h."""
                            scab = q0ps.tile([128, 1024], bf16 if SCAB_BF16 else f32,
                                             tag=f"q0s{h}",
                                             bufs=SCAB_BUFS if SCAB_BF16 else 1)
                            for j, kb in enumerate((2 * p, 2 * p + 1)):
                                score_mm(scab[:, 512 * j:512 * (j + 1)], h, kb,
                                         slice(0, 512))
                            nc.scalar.activation(q0eab[2 * p + h][:], scab[:],
                                                 Exp, scale=0.125)

                        def load_xc(nt):
                            # host layout is nt-major: one contiguous descriptor
                            xc = p1sb.tile([128, 8 * 512], f8, tag="xc", bufs=2)
                            nc.sync.dma_start(xc[:],
                                              xTb_d[:, 4096 * nt:4096 * (nt + 1)])
                            return xc

                        xc_next = load_xc(0)
                        for nt in range(NT):
                            xc = xc_next
                            if nt + 1 < NT:
                                xc_next = load_xc(nt + 1)
                            chunks = ([(p, h) for p in (2 * nt - 2, 2 * nt - 1)
                                       for h in (0, 1)] if nt > 0 else [])
                            ps = {}
                            for gi, pname in enumerate(("wq", "wk", "wv")):
                                if gi < len(chunks):
                                    q0_chunk(*chunks[gi])
                                p_t = p1ps.tile([128, 512], f32, tag=f"{pname}ps")
                                for k2 in range(4):
                                    nc.tensor.matmul(
                                        p_t[:],
                                        w_all[pname][:, 256 * k2:256 * (k2 + 1)]
                                            .rearrange("p (i o) -> p i o", i=2),
                                        xc[:, 1024 * k2:1024 * (k2 + 1)]
                                            .rearrange("p (i n) -> p i n", i=2),
                                        start=(k2 == 0), stop=(k2 == 3), perf_mode=DR)
                                ps[pname] = p_t
                            sl = slice(512 * nt, 512 * (nt + 1))
                            nc.vector.tensor_scalar_mul(QT[:, sl], ps["wq"][:], 1.0 / WS)
                            if SCORE_K128:
                                nc.vector.tensor_scalar_mul(
                                    KT[0][0:64, sl], ps["wk"][0:64, :], 1.0 / WS)
                                nc.vector.tensor_scalar_mul(
                                    KT[1][64:128, sl], ps["wk"][64:128, :], 1.0 / WS)
                            else:
                                nc.vector.tensor_scalar_mul(KT[0][:, sl], ps["wk"][:],
                                                            1.0 / WS)
                            VT = p1sb.tile([128, 512], bf16, tag="VT", bufs=2)
                            nc.vector.tensor_scalar_mul(VT[:], ps["wv"][:], 1.0 / WS)
                            tp = trps.tile([128, 512], bf16, tag="tp")
                            for b in range(4):
                                nc.tensor.transpose(
                                    tp[:, 128 * b:128 * (b + 1)],
                                    VT[:, 128 * b:128 * (b + 1)], idbf_t[:])
                            # tp block b (kb=4nt+b) cols [64h:64h+64] -> VB col
                            # 160*(2*(kb//2)+h) + 80*(kb%2); per h two 2-block
                            # strided copies (pairs (0,1) and (2,3))
                            for h in range(2):
                                for g in range(2):
                                    base = 2 * VBW * (4 * nt + 2 * g + h)
                                    nc.vector.tensor_copy(
                                        VB[:, base:base + 2 * VBW]
                                          .rearrange("p (b e) -> p b e", b=2)[:, :, 0:64],
                                        tp[:, 256 * g:256 * (g + 1)]
                                          .rearrange("p (b e) -> p b e", b=2)[:, :, 64 * h:64 * h + 64])
                            if len(chunks) > 3:
                                q0_chunk(*chunks[3])
                        for p in (2 * NT - 2, 2 * NT - 1):
                            for h in range(2):
                                q0_chunk(p, h)

                    if phases == 1:   # debug: dump QT/KT/VB + q0eab[0:4]
                        nc.sync.dma_start(out_d[0:128, :], QT[:, 0:2048].bitcast(f32))
                        nc.sync.dma_start(out_d[128:256, :], KT[0][:, 0:2048].bitcast(f32))
                        nc.sync.dma_start(out_d[256:384, :], VB[:, 0:4096].bitcast(f32))
                        for j in range(4):
                            nc.sync.dma_start(out_d[384:512, 256 * j:256 * (j + 1)],
                                              q0eab[j][:].bitcast(f32))
                        return

                    # ================= P2: qt1..7 + qt0 burst + P3 ================
                    with ExitStack() as c2:
                        if phases >= 3:
                            load_p3_consts()
                        atps = c2.enter_context(tc.tile_pool(name="atps", bufs=1, space="PSUM"))
                        scps = c2.enter_context(tc.tile_pool(name="scps", bufs=1, space="PSUM"))
                        mlps = c2.enter_context(tc.tile_pool(name="mlps", bufs=2, space="PSUM"))

                        attps0 = [atps.tile([65, 512], f32, name=f"attps{h}")
                                  for h in range(2)]
                        # qt0 att accumulators, carved from the mlp ring (mlp
                        # mms only start at qt2)
                        q0acc = [mlps.tile([128, 512], f32, tag="mp",
                                           name=f"q0acc{h}") for h in range(2)]

                        def ln_stats(state, key):
                            """aggr + bit-trick rsqrt from state[key+'st6']."""
                            ag = p3sb.tile([128, 2], f32, tag="ag", bufs=2)
                            nc.vector.bn_aggr(ag[:], state[key + "st6"][:])
                            nmu = p3sb.tile([128, 1], f32, tag="nmu", bufs=2)
                            nc.vector.tensor_scalar_mul(nmu[:], ag[:, 0:1], -1.0)
                            ve = p3sb.tile([128, 1], f32, tag="ve", bufs=2)
                            nc.vector.tensor_scalar_add(ve[:], ag[:, 1:2], EPS)
                            inv = p3sb.tile([128, 1], f32, tag="inv", bufs=2)
                            ts = p3sb.tile([128, 1], f32, tag="ts", bufs=2)
                            us = p3sb.tile([128, 1], f32, tag="us", bufs=2)
                            rsqrt_dve(inv, ve, ts, us)
                            state[key + "nmu"], state[key + "inv"] = nmu, inv

                        def p3_block(r):
                            """P3 for row block r: (head_chunk, rest_chunks)."""
                            state = {}
                            xres_r = xres_all[:, D * r:D * (r + 1)]

                            def c_s1a():   # runs one qtile early (half 0 only)
                                s1 = p3sb.tile([128, D], f32, tag="s1", bufs=2)
                                st6 = p3sb.tile([128, 2, 6], f32, tag="st6a", bufs=2)
                                nc.vector.tensor_add(s1[:, 0:512], att_perm[r][:, 0:512],
                                                     xres_r[:, 0:512])
                                nc.vector.bn_stats(st6[:, 0, :], s1[:, 0:512])
                                state["s1"], state["ast6"] = s1, st6

                            def c_s1b():
                                s1 = state["s1"]
                                nc.vector.tensor_add(s1[:, 512:1024],
                                                     att_perm[r][:, 512:1024],
                                                     xres_r[:, 512:1024])
                                nc.vector.bn_stats(state["ast6"][:, 1, :], s1[:, 512:1024])
                                ln_stats(state, "a")
                                state["h1"] = p3sb.tile([128, D], bf16, tag="h1", bufs=2,
                                                        name="h1")
                                state["h1T"] = p3sb.tile([128, D], bf16, tag="h1T", bufs=2,
                                                         name="h1T")
                                state["s2"] = p3sb.tile([128, D], f32, tag="s2", bufs=2,
                                                        name="s2")
                                state["bst6"] = p3sb.tile([128, 2, 6], f32, tag="st6b",
                                                          bufs=2, name="bst6")

                            def c_nh(j):   # LN1 apply + transpose, half j
                                s1, h1 = state["s1"], state["h1"]
                                sl = slice(512 * j, 512 * (j + 1))
                                nc.vector.scalar_tensor_tensor(
                                    s1[:, sl], s1[:, sl], state["anmu"], P_G1[:, sl],
                                    op0=Alu.add, op1=Alu.mult)
                                nc.vector.scalar_tensor_tensor(
                                    h1[:, sl], s1[:, sl], state["ainv"], P_BB1[:, sl],
                                    op0=Alu.mult, op1=Alu.add)
                                nc.sync.dma_start(
                                    state["h1T"][:, sl].rearrange("p (cb r2) -> p cb r2", cb=4),
                                    h1[:, sl], transpose=True)

                            def c_mlp(jt, half):   # 4 matmuls; finishes s2 half jt
                                jsl = slice(512 * jt, 512 * (jt + 1))
                                if half == 0:
                                    state[f"mp{jt}"] = mlps.tile([128, 512], f32, tag="mp",
                                                                 name=f"mp{jt}")
                                mp = state[f"mp{jt}"]
                                # deprioritized: PE runs these only when the
                                # score/att stream has nothing ready
                                prio0 = tc.cur_priority
                                tc.cur_priority += MLP_PRIO
                                for cb in range(4 * half, 4 * half + 4):
                                    nc.tensor.matmul(
                                        mp[:], state["h1T"][:, 128 * cb:128 * (cb + 1)],
                                        w1_all[:, D * cb + 512 * jt:D * cb + 512 * (jt + 1)],
                                        start=(cb == 0), stop=(cb == 7))
                                tc.cur_priority = prio0
                                if half == 1:
                                    s2 = state["s2"]
                                    nc.vector.scalar_tensor_tensor(
                                        s2[:, jsl], mp[:], 1.0, P_B1[:, jsl],
                                        op0=Alu.mult, op1=Alu.add)
                                    nc.vector.tensor_add(s2[:, jsl], s2[:, jsl],
                                                         state["h1"][:, jsl])
                                    nc.vector.bn_stats(state["bst6"][:, jt, :], s2[:, jsl])

                            def c_f(j):   # LN2 apply + store, half j
                                if j == 0:
                                    ln_stats(state, "b")
                                    state["ot"] = p3sb.tile([128, D], f32, tag="ot",
                                                            bufs=2, name="ot")
                                s2, o_t = state["s2"], state["ot"]
                                sl = slice(512 * j, 512 * (j + 1))
                                nc.vector.scalar_tensor_tensor(
                                    s2[:, sl], s2[:, sl], state["bnmu"], P_G2[:, sl],
                                    op0=Alu.add, op1=Alu.mult)
                                nc.vector.scalar_tensor_tensor(
                                    o_t[:, sl], s2[:, sl], state["binv"], P_BB2[:, sl],
                                    op0=Alu.mult, op1=Alu.add)
                                nc.sync.dma_start(out_d[128 * r:128 * (r + 1), sl],
                                                  o_t[:, sl])

                            return c_s1a, [c_s1b, lambda: c_nh(0), lambda: c_nh(1),
                                           lambda: c_mlp(0, 0), lambda: c_mlp(0, 1),
                                           lambda: c_mlp(1, 0), lambda: c_mlp(1, 1),
                                           lambda: c_f(0), lambda: c_f(1)]

                        p3_blocks = [p3_block(r) for r in range(4)] if phases >= 3 else None
                        p3_queue = []
                        carry = [None]

                        def run_carry():
                            if carry[0] is not None:
                                carry[0]()
                                carry[0] = None

                        for qt in range(1, NT):
                            qsl = slice(512 * qt, 512 * (qt + 1))
                            if phases >= 3 and qt % 2 == 1:
                                p3_queue.append(p3_blocks[(qt - 1) // 2][0])
                            pend = []
                            for bi in range(16):
                                eabs = []
                                for h in range(2):
                                    scab = scps.tile([128, 1024],
                                                     bf16 if SCAB_BF16 else f32,
                                                     tag=f"scab{h}", name=f"scab{h}",
                                                     bufs=SCAB_BUFS if SCAB_BF16 else 1)
                                    for j, kb in enumerate((2 * bi, 2 * bi + 1)):
                                        score_mm(scab[:, 512 * j:512 * (j + 1)],
                                                 h, kb, qsl)
                                    eab = p2sb.tile([128, 1024], f8, tag=f"eab{h}",
                                                    name=f"eab{h}", bufs=6)
                                    if qt >= 2 and (bi, h) in dve_bis:
                                        nc.vector.tensor_scalar(
                                            eab[:].bitcast(i8), scab[:], A8P, B8P,
                                            op0=Alu.mult, op1=Alu.add)
                                    else:
                                        nc.scalar.activation(eab[:], scab[:], Exp,
                                                             scale=0.125)
                                    eabs.append(eab)
                                if bi == 0:
                                    run_carry()   # prev qtile's last att + normalize
                                # atts trail exps by 2 bi so the carry's
                                # normalize gets 2 bi to release attps; late in
                                # the last qtile pop TWO per bi so only one att
                                # batch remains after the final exp (shorter
                                # kernel tail; doing this for EVERY qtile
                                # measured no better and couples atts tighter
                                # to exps at qtile ends)
                                pops = 2 if (qt == NT - 1 and bi >= 14) else 1
                                for _ in range(pops):
                                    if len(pend) >= (2 if pops == 1 else 1):
                                        pb, peabs = pend.pop(0)
                                        for h in range(2):
                                            att_dr(attps0[h][0:65, :], h, pb,
                                                   peabs[h], start=(pb == 0),
                                                   stop=(pb == 15))
                                pend.append((bi, eabs))
                                if qt == 1:
                                    if bi < 8:   # qt0 att burst: pairs 2bi, 2bi+1
                                        # deprioritized: fills PE gaps in qt1's
                                        # own stream; must finish by bi8
                                        prio0 = tc.cur_priority
                                        tc.cur_priority += 300
                                        for p in (2 * bi, 2 * bi + 1):
                                            for h in range(2):
                                                att_dr(q0acc[h][0:65, :], h, p,
                                                       q0eab[2 * p + h],
                                                       start=(p == 0), stop=(p == 15))
                                        tc.cur_priority = prio0
                                    elif bi == 8:
                                        normalize(q0acc, 0, 0)
                                    elif bi == 11 and phases >= 3 and p3_queue:
                                        p3_queue.pop(0)()
                                elif p3_queue and bi in (2, 4, 6, 8, 10, 12):
                                    p3_queue.pop(0)()

                            def qt_tail(pend=pend, qt=qt):
                                for pb, peabs in pend:
                                    for h in range(2):
                                        att_dr(attps0[h][0:65, :], h, pb, peabs[h],
                                               start=(pb == 0), stop=(pb == 15))
                                normalize(attps0, qt // 2, 512 * (qt % 2))
                            carry[0] = qt_tail
                            if phases >= 3 and qt % 2 == 1:
                                p3_queue.extend(p3_blocks[(qt - 1) // 2][1])
                        run_carry()
                        if phases == 2:   # debug: dump att_perm
                            for r in range(4):
                                nc.sync.dma_start(out_d[128 * r:128 * (r + 1), :],
                                                  att_perm[r][:])
                            return
                        # drain remaining P3 work (r=3)
                        while p3_queue:
                            p3_queue.pop(0)()

            if timing_reps:
                for _rep in range(timing_reps):
                    body()
            elif loop:
                with tc.For_i(0, loop, 1, staggered_reset=True,
                              hint_engines=mybir.ALL_ENGINES) as _:
                    body()
            else:
                body()
            if tick_d is not None:
                tick = main.tile([1, 4], f32)
                nc.vector.tensor_copy(tick[:], QT[0:1, 0:8].bitcast(f32))
                nc.sync.dma_start(tick_d[:], tick[:])
    nc.compile()
    return nc


_CACHE = {}


def _get_nc(loop=0, phases=3, timing_reps=0, internal=False, dve_bis=None):
    key = (loop, phases, timing_reps, internal, dve_bis)
    if key not in _CACHE:
        _CACHE[key] = build(loop, phases, timing_reps, internal, dve_bis)
    return _CACHE[key]


def make_in_maps(x, wq, wk, wv, ln1_g, ln1_b, w1, b1, ln2_g, ln2_b):
    import ml_dtypes
    x = np.asarray(x, np.float32)
    xT = np.ascontiguousarray(x.T)  # [D, N]
    # nt-major: xTb[p, 4096*nt + 512*k + c] = xT[128k+p, 512nt+c], so each
    # per-nt xc load is one contiguous [128, 4096] descriptor
    xTb = np.ascontiguousarray(
        xT.reshape(8, 128, 8, 512).transpose(1, 2, 0, 3).reshape(128, 8 * N)
    ).astype(ml_dtypes.float8_e4m3)

    def w_pre(w, rs):
        wT = np.ascontiguousarray(np.asarray(w, np.float32)[rs].T * 64.0)  # [D,128]
        return np.ascontiguousarray(
            wT.reshape(8, 128, 128).transpose(1, 0, 2).reshape(128, D)
        ).astype(ml_dtypes.float8_e4m3)
    w1T = np.ascontiguousarray(np.asarray(w1, np.float32).T).astype(ml_dtypes.bfloat16)
    bcast = lambda v: np.broadcast_to(np.asarray(v, np.float32), (128, D))
    prm = np.ascontiguousarray(np.concatenate(
        [bcast(b1), bcast(ln1_g), bcast(ln1_b), bcast(ln2_g), bcast(ln2_b)], axis=1))
    ones64 = np.ones((1, 64), np.float32)
    idbf = np.eye(128, dtype=ml_dtypes.bfloat16)
    in_maps = []
    for c in range(8):
        rs = slice(128 * c, 128 * (c + 1))
        in_maps.append({
            "xTb": xTb,
            "wqT": w_pre(wq, rs),
            "wkT": w_pre(wk, rs),
            "wvT": w_pre(wv, rs),
            "w1T": w1T,
            "xres": np.ascontiguousarray(x[512 * c:512 * (c + 1)][_PERM]),
            "prm": prm,
            "ones64": ones64, "idbf": idbf,
        })
    return in_maps


def kernel(**inputs):
    nc = _get_nc(0)
    in_maps = make_in_maps(**inputs)
    results = bass2jax.run_bass_via_pjrt(nc, in_maps, n_cores=8)
    outs = []
    for c in range(8):
        o = np.empty((ROWS, D), np.float32)
        o[_PERM] = results[c]["out"]
        outs.append(o)
    return np.concatenate(outs, axis=0).astype(np.float32)
